# revision 1
# baseline (speedup 1.0000x reference)
"""Delta-memory scan kernel for TRN2 (Bass/Tile) - self-contained.

Layout summary (per core, one batch element):
  - chunk T=64 steps; solve state at partition 0; Dsc matrix maintained in
    PSUM via PE one-hot scatter, snapshotted to SBUF by ACT (2 rotating bufs)
  - per step: 2-3 trailing stt AXPYs + fused sq-reduce + pow(-0.5) + max + scale
    (all DVE, the critical chain)
  - PE: projections, A=KK^T, bulk corrections, scatters, outputs (off-chain)
"""
import sys
sys.path.insert(0, "/opt/trn_rl_repo")
import numpy as np
import concourse.bass as bass
import concourse.bacc as bacc
import concourse.mybir as mybir
from concourse.tile import TileContext

F32 = mybir.dt.float32
OP = mybir.AluOpType
HIDDEN = 1024
MEM = 64
CLIP = 50.0
T = 64  # chunk length


def host_consts():
    """Constant input tensors shared by all cores."""
    eye = np.eye(MEM, dtype=np.float32)
    iflat = eye.reshape(1, MEM * MEM).copy()
    e_masks = np.zeros((4, MEM, MEM), np.float32)  # w=0..3 superdiag masks (+1)
    for w in range(4):
        for s in range(MEM - w):
            e_masks[w, s, s + w] = 1.0
    masku_neg = -np.triu(np.ones((T, T), np.float32))  # -(s<=t)
    ones_row = np.ones((1, MEM), np.float32)
    ones_col = np.ones((MEM, 1), np.float32)
    neg_i64 = -eye
    return {
        "iflat": iflat, "negi": neg_i64, "i64": eye,
        "em0": e_masks[0], "em1": e_masks[1], "em2": e_masks[2], "em3": e_masks[3],
        "masku": masku_neg, "onesr": ones_row, "onesc": ones_col,
        "zrow": np.zeros((1, MEM), np.float32),
        "i128": np.eye(128, dtype=np.float32),
    }


def build(nc, L):
    NCH = L // T
    d = {}
    # ---- dram tensors ----
    x = nc.dram_tensor("x", [L, HIDDEN], F32, kind="ExternalInput").ap()
    wq = nc.dram_tensor("wq", [HIDDEN, MEM], F32, kind="ExternalInput").ap()
    wk = nc.dram_tensor("wk", [HIDDEN, MEM], F32, kind="ExternalInput").ap()
    wv = nc.dram_tensor("wv", [HIDDEN, MEM], F32, kind="ExternalInput").ap()
    wo = nc.dram_tensor("wo", [MEM, HIDDEN], F32, kind="ExternalInput").ap()
    bo = nc.dram_tensor("bo", [1, HIDDEN], F32, kind="ExternalInput").ap()
    consts = nc.dram_tensor("consts", [1, 8], F32, kind="ExternalInput").ap()
    hk = host_consts()
    hin = {}
    for name, arr in hk.items():
        hin[name] = nc.dram_tensor(name, list(arr.shape), F32, kind="ExternalInput").ap()
    y = nc.dram_tensor("y", [L, HIDDEN], F32, kind="ExternalOutput").ap()
    fmem = nc.dram_tensor("fmem", [MEM, MEM], F32, kind="ExternalOutput").ap()

    with TileContext(nc) as tc:
        with (
            tc.tile_pool(name="persist", bufs=1) as pp,
            tc.tile_pool(name="chunk", bufs=2) as cp,
            tc.tile_pool(name="step", bufs=2) as sp,
            tc.tile_pool(name="ps_seq", bufs=2, space="PSUM") as ps_seq,
            tc.tile_pool(name="ps_sd", bufs=1, space="PSUM") as ps_sd,
            tc.tile_pool(name="ps_slot", bufs=1, space="PSUM") as ps_slot,
            tc.tile_pool(name="ps_a", bufs=2, space="PSUM") as ps_a,
            tc.tile_pool(name="ps_b", bufs=1, space="PSUM") as ps_b,
        ):
            # ---- persistent tiles ----
            wq_sb = pp.tile([128, 8 * MEM], F32, tag="wq")
            wk_sb = pp.tile([128, 8 * MEM], F32, tag="wk")
            wv_sb = pp.tile([128, 8 * MEM], F32, tag="wv")
            wo_sb = pp.tile([MEM, HIDDEN], F32, tag="wo")
            iflat_sb = pp.tile([1, MEM * MEM], F32, tag="iflat")
            negi_sb = pp.tile([MEM, MEM], F32, tag="negi")
            i64_sb = pp.tile([MEM, MEM], F32, tag="i64")
            em_sb = [pp.tile([MEM, MEM], F32, tag=f"em{w}", name=f"em{w}") for w in range(4)]
            masku_sb = pp.tile([T, T], F32, tag="masku")
            onesr_sb = pp.tile([1, MEM], F32, tag="onesr")
            onesc_sb = pp.tile([MEM, 1], F32, tag="onesc")
            zrow_sb = pp.tile([1, MEM], F32, tag="zrow")
            consts_sb = pp.tile([1, 8], F32, tag="consts")
            bosc_sb = pp.tile([MEM, HIDDEN], F32, tag="bosc")
            scol_sb = pp.tile([MEM, 1], F32, tag="scol")
            memt_sb = pp.tile([MEM, MEM], F32, tag="memt")
            aaug_sb = pp.tile([128, T], F32, tag="aaug")
            i128_sb = pp.tile([128, 128], F32, tag="i128")
            snap_sb = [pp.tile([128, MEM], F32, tag=f"snap{i}", name=f"snap{i}") for i in range(2)]
            ring = [pp.tile([1, MEM], F32, tag=f"ring{i}", name=f"ring{i}") for i in range(4)]
            rsc = [pp.tile([MEM, 1], F32, tag=f"rsc{i}", name=f"rsc{i}") for i in range(4)]
            ones1_sb = pp.tile([1, 1], F32, tag="ones1")

            # ---- load constants ----
            for ap_, t_ in ((wq, wq_sb), (wk, wk_sb), (wv, wv_sb)):
                nc.sync.dma_start(t_[:].rearrange("p (a m) -> p a m", a=8),
                                  ap_.rearrange("(a p) m -> p a m", p=128))
            nc.sync.dma_start(wo_sb[:], wo[:])
            nc.sync.dma_start(iflat_sb[:], hin["iflat"][:])
            nc.sync.dma_start(negi_sb[:], hin["negi"][:])
            nc.sync.dma_start(i64_sb[:], hin["i64"][:])
            for w in range(4):
                nc.sync.dma_start(em_sb[w][:], hin[f"em{w}"][:])
            nc.sync.dma_start(masku_sb[:], hin["masku"][:])
            nc.sync.dma_start(onesr_sb[:], hin["onesr"][:])
            nc.sync.dma_start(onesc_sb[:], hin["onesc"][:])
            nc.sync.dma_start(zrow_sb[:], hin["zrow"][:])
            nc.sync.dma_start(consts_sb[:], consts[:])
            nc.sync.dma_start(aaug_sb[MEM:128, :], hin["i64"][:])  # identity part
            nc.sync.dma_start(i128_sb[:], hin["i128"][:])

            NEGBETA = consts_sb[0:1, 0:1]
            NEGBC = consts_sb[0:1, 1:2]
            NEGBC2 = consts_sb[0:1, 1:2]
            SCALE = consts_sb[0:1, 2:3]

            for r in ring:
                nc.vector.memset(r[:], 0.0)
            for r in rsc:
                nc.vector.memset(r[:], 0.0)
            nc.vector.memset(ones1_sb[:], 1.0)
            nc.vector.memset(memt_sb[:], 0.0)

            # bosc = scale * bo broadcast to 64 partitions; scol = scale column
            bor = pp.tile([1, HIDDEN], F32, tag="bor")
            nc.sync.dma_start(bor[:], bo[:])
            bos = pp.tile([1, HIDDEN], F32, tag="bos")
            nc.vector.tensor_scalar(bos[:], bor[:], SCALE, None, OP.mult)
            for h in range(2):
                pb = ps_seq.tile([MEM, 512], F32, tag="seq")
                nc.tensor.matmul(pb[:], onesr_sb[:], bos[0:1, h * 512:(h + 1) * 512],
                                 start=True, stop=True)
                nc.scalar.copy(bosc_sb[:, h * 512:(h + 1) * 512], pb[:])
            pscol = ps_seq.tile([MEM, 1], F32, tag="seq")
            nc.tensor.matmul(pscol[:], onesr_sb[:], SCALE, start=True, stop=True)
            nc.scalar.copy(scol_sb[:], pscol[:])

            pb_t = [ps_b.tile([MEM, 1], F32, tag=f"pb{i}", name=f"pb{i}")
                    for i in range(2)]
            # initialize pb psums (read x0 by first-chunk trailing stts)
            for i in range(2):
                nc.tensor.matmul(pb_t[i][:], zrow_sb[:], ones1_sb[:],
                                 is_transpose=True, start=True, stop=True)
            # ---- chunk loop ----
            for c in range(NCH):
                xc = cp.tile([T, HIDDEN], F32, tag="xc")
                nc.sync.dma_start(xc[:], x[c * T:(c + 1) * T, :])
                # transpose x chunk: 8 x [64,128] -> [128,64]
                xt = cp.tile([128, 8 * MEM], F32, tag="xt")
                for a in range(8):
                    ptp = ps_seq.tile([128, T], F32, tag="seq")
                    nc.tensor.transpose(ptp[:], xc[:, a * 128:(a + 1) * 128], i64_sb[:])
                    nc.scalar.copy(xt[:, a * MEM:(a + 1) * MEM], ptp[:])
                # projections
                kt_sb = cp.tile([MEM, T], F32, tag="kt")
                qt_sb = cp.tile([MEM, T], F32, tag="qt")
                v_sb = cp.tile([T, MEM], F32, tag="v")
                negk_sb = cp.tile([T, MEM], F32, tag="negk")
                projs = [
                    (wk_sb, True, kt_sb, 1.0), (wq_sb, True, qt_sb, 1.0),
                    (wv_sb, False, v_sb, 1.0), (wk_sb, False, negk_sb, -1.0),
                ]
                for wsb, tform, dst, scl in projs:
                    pp_ = ps_seq.tile([MEM if tform else T, T if tform else MEM],
                                      F32, tag="seq", name="pproj")
                    for a in range(8):
                        sl = slice(a * MEM, (a + 1) * MEM)
                        if tform:
                            nc.tensor.matmul(pp_[:], wsb[:, sl], xt[:, sl],
                                             start=(a == 0), stop=(a == 7))
                        else:
                            nc.tensor.matmul(pp_[:], xt[:, sl], wsb[:, sl],
                                             start=(a == 0), stop=(a == 7))
                    nc.scalar.mul(dst[:], pp_[:], scl)
                # A = K K^T -> aaug rows 0:64
                pa = ps_seq.tile([T, T], F32, tag="seq")
                nc.tensor.matmul(pa[:], kt_sb[:], kt_sb[:], start=True, stop=True)
                nc.scalar.mul(aaug_sb[0:T, :], pa[:], -1.0)
                # superdiag rows: am_w = A*Ew ; arow[w] = ones^T @ am_w
                parow = ps_seq.tile([1, 4 * T], F32, tag="seq")
                am = cp.tile([T, 4 * T], F32, tag="am")
                for w in range(4):
                    nc.vector.tensor_tensor(am[:, w * T:(w + 1) * T], pa[:], em_sb[w][:],
                                            OP.mult)
                    nc.tensor.matmul(parow[0:1, w * T:(w + 1) * T], onesc_sb[:],
                                     am[:, w * T:(w + 1) * T], start=True, stop=True)
                arow = cp.tile([1, 4 * T], F32, tag="arow")
                nc.scalar.copy(arow[:], parow[:])
                pnab = ps_seq.tile([MEM, 4 * T], F32, tag="seq", name="pnab")
                nc.tensor.matmul(pnab[:], onesr_sb[:], arow[:], start=True, stop=True)
                nab = cp.tile([MEM, 4 * T], F32, tag="nab")
                nc.scalar.copy(nab[:], pnab[:])
                # init psum: rows 64:128 = -V + K@memT0 ; rows 0:64 zeroed
                psd0 = ps_seq.tile([128, MEM], F32, tag="seq", name="psd0")
                nc.tensor.matmul(psd0[MEM:128, :], negi_sb[:], v_sb[:], start=True,
                                 stop=False)
                nc.tensor.matmul(psd0[MEM:128, :], kt_sb[:], memt_sb[:], start=False,
                                 stop=True)
                nc.tensor.matmul(psd0[0:MEM, :], iflat_sb[0:1, 0:MEM], zrow_sb[:],
                                 start=True, stop=True)
                nc.scalar.copy(snap_sb[0][:], psd0[:])
                nc.scalar.copy(snap_sb[1][:], psd0[:])

                # ---- solve ----
                for g in range(T // 2):
                    t0 = 2 * g
                    snap = snap_sb[g % 2]
                    psdg = ps_sd.tile([MEM, MEM], F32, tag="sd", name="psdg")
                    nc.tensor.matmul(psdg[:], i128_sb[0:MEM, 0:MEM],
                                     snap_sb[(g - 1) % 2][0:MEM, :],
                                     start=True, stop=False)
                    slot = ps_slot.tile([MEM, 2], F32, tag="slot", name="slot")
                    nc.tensor.matmul(slot[:, 0:1], snap[:], aaug_sb[:, t0:t0 + 1],
                                     start=True, stop=False)
                    nc.tensor.matmul(slot[:, 1:2], snap[:], aaug_sb[:, t0 + 1:t0 + 2],
                                     start=False, stop=True)
                    for t in (t0, t0 + 1):
                        ti = t % 4
                        slot_c = slot[:, 0:1] if t == t0 else slot[:, 1:2]
                        delta = sp.tile([MEM, 1], F32, tag="delta")
                        tmp = sp.tile([MEM, 1], F32, tag="tmp")
                        if t == t0:  # w=2 (sbuf rsc), w=1 (psum direct)
                            nc.vector.scalar_tensor_tensor(
                                out=tmp[:], in0=rsc[(t - 2) % 4][:],
                                scalar=nab[:, 2 * T + t:2 * T + t + 1],
                                in1=slot_c, op0=OP.mult, op1=OP.subtract)
                            nc.vector.scalar_tensor_tensor(
                                out=delta[:], in0=pb_t[(t - 1) % 2][:],
                                scalar=nab[:, T + t:T + t + 1],
                                in1=tmp[:], op0=OP.mult, op1=OP.add)
                        else:  # w=3 (sbuf), w=2 (psum), w=1 (psum)
                            tmp2 = sp.tile([MEM, 1], F32, tag="tmp2")
                            nc.vector.scalar_tensor_tensor(
                                out=tmp[:], in0=rsc[(t - 3) % 4][:],
                                scalar=nab[:, 3 * T + t:3 * T + t + 1],
                                in1=slot_c, op0=OP.mult, op1=OP.subtract)
                            nc.vector.scalar_tensor_tensor(
                                out=tmp2[:], in0=pb_t[(t - 2) % 2][:],
                                scalar=nab[:, 2 * T + t:2 * T + t + 1],
                                in1=tmp[:], op0=OP.mult, op1=OP.add)
                            nc.vector.scalar_tensor_tensor(
                                out=delta[:], in0=pb_t[(t - 1) % 2][:],
                                scalar=nab[:, T + t:T + t + 1],
                                in1=tmp2[:], op0=OP.mult, op1=OP.add)
                        pa_t = ps_a.tile([1, 66], F32, tag="pa", name="pa_t")
                        nc.tensor.matmul(pa_t[0:1, 64:65], delta[:], delta[:],
                                         start=True, stop=False)
                        nc.tensor.matmul(pa_t[0:1, 0:64], delta[:], i64_sb[:],
                                         is_transpose=True, start=False, stop=True)
                        un = sp.tile([1, 1], F32, tag="un")
                        nc.scalar.activation(un[:], pa_t[0:1, 64:65],
                                             mybir.ActivationFunctionType.Sqrt,
                                             scale=arow[0:1, t:t + 1])
                        um = sp.tile([1, 1], F32, tag="um")
                        nc.vector.tensor_scalar(um[:], un[:], CLIP, None, OP.max)
                        rr = sp.tile([1, 1], F32, tag="rr")
                        nc.vector.reciprocal(rr[:], um[:])
                        ssc = sp.tile([1, 1], F32, tag="ssc")
                        nc.vector.tensor_scalar(ssc[:], rr[:], NEGBC2, None, OP.mult)
                        rrow = ring[ti]
                        nc.scalar.activation(rrow[:], pa_t[0:1, 0:64],
                                             mybir.ActivationFunctionType.Copy,
                                             scale=ssc[0:1, 0:1])
                        nc.tensor.matmul(pb_t[t % 2][:], rrow[:], ones1_sb[:],
                                         is_transpose=True, start=True, stop=True)
                        if t % 2 == 0:  # sbuf copy only needed for w2/w3 reads
                            nc.scalar.copy(rsc[ti][:], pb_t[t % 2][:])
                        nc.tensor.matmul(psdg[:],
                                         iflat_sb[0:1, t * MEM:(t + 1) * MEM],
                                         rrow[:], start=False, stop=(t == t0 + 1))
                    nc.scalar.copy(snap_sb[g % 2][0:MEM, :], psdg[:])
                dsc = snap_sb[(T // 2 - 1) % 2]  # final snapshot holds all rows
                # outputs
                pst = ps_seq.tile([T, T], F32, tag="seq")
                nc.tensor.matmul(pst[:], kt_sb[:], qt_sb[:], start=True, stop=True)
                smt = cp.tile([T, T], F32, tag="smt")
                nc.vector.tensor_tensor(smt[:], pst[:], masku_sb[:], OP.mult)
                pout = ps_seq.tile([MEM, T], F32, tag="seq")
                nc.tensor.matmul(pout[:], memt_sb[:], qt_sb[:], start=True, stop=False)
                nc.tensor.matmul(pout[:], dsc[0:MEM, :], smt[:], start=False, stop=True)
                outt = cp.tile([MEM, T], F32, tag="outt")
                nc.scalar.copy(outt[:], pout[:])
                ych = cp.tile([T, HIDDEN], F32, tag="ych")
                for h in range(2):
                    hs = slice(h * 512, (h + 1) * 512)
                    py = ps_seq.tile([T, 512], F32, tag="seq")
                    nc.tensor.matmul(py[:], outt[:], wo_sb[:, hs], start=True, stop=True)
                    nc.vector.scalar_tensor_tensor(
                        out=ych[:, hs], in0=py[:], scalar=scol_sb[:],
                        in1=bosc_sb[:, hs], op0=OP.mult, op1=OP.add)
                nc.sync.dma_start(y[c * T:(c + 1) * T, :], ych[:])
                # memT update
                pm = ps_seq.tile([MEM, MEM], F32, tag="seq")
                nc.tensor.matmul(pm[:], i64_sb[:], memt_sb[:], start=True, stop=False)
                nc.tensor.matmul(pm[:], negk_sb[:], dsc[0:MEM, :], start=False,
                                 stop=True)
                nc.scalar.copy(memt_sb[:], pm[:])

            # final mem output (transpose memT)
            pfm = ps_seq.tile([MEM, MEM], F32, tag="seq")
            nc.tensor.transpose(pfm[:], memt_sb[:], i64_sb[:])
            fm_sb = pp.tile([MEM, MEM], F32, tag="fm")
            nc.scalar.copy(fm_sb[:], pfm[:])
            nc.sync.dma_start(fmem[:], fm_sb[:])
    return nc


def make_inputs(xb, Wq, Wk, Wv, Wo, bo_, beta, scale):
    hk = host_consts()
    consts = np.zeros((1, 8), np.float32)
    consts[0, 0] = -beta
    consts[0, 1] = -beta * CLIP
    consts[0, 2] = scale
    inp = {
        "x": np.ascontiguousarray(xb, np.float32),
        "wq": np.ascontiguousarray(Wq, np.float32),
        "wk": np.ascontiguousarray(Wk, np.float32),
        "wv": np.ascontiguousarray(Wv, np.float32),
        "wo": np.ascontiguousarray(Wo, np.float32),
        "bo": np.ascontiguousarray(bo_.reshape(1, HIDDEN), np.float32),
        "consts": consts,
    }
    inp.update(hk)
    return inp


# ---------------------------------------------------------------------------
# Public entry point: full-input kernel with internal 8-core SPMD dispatch.
# Batch b of 4 runs on cores b and b+4 (duplicated); outputs taken from 0-3.
# ---------------------------------------------------------------------------
_COMPILED = {}


def _get_compiled(L):
    if L not in _COMPILED:
        nc = bacc.Bacc("TRN2", target_bir_lowering=False, debug=False,
                       num_devices=8)
        build(nc, L)
        nc.compile()
        _COMPILED[L] = nc
    return _COMPILED[L]


def kernel(x, Wq, Wk, Wv, Wo, bo, memory_scale, beta_param):
    from concourse import bass_utils
    x = np.ascontiguousarray(np.asarray(x), dtype=np.float32)
    B, L, H = x.shape
    assert H == HIDDEN
    beta = 0.1 + (1.0 / (1.0 + np.exp(-float(np.asarray(beta_param).reshape(-1)[0])))) * 0.8
    scale = abs(float(np.asarray(memory_scale).reshape(-1)[0]))
    nc = _get_compiled(L)
    in_maps = []
    for c in range(8):
        in_maps.append(make_inputs(x[c % B], Wq, Wk, Wv, Wo,
                                   np.asarray(bo), beta, scale))
    res = bass_utils.run_bass_kernel_spmd(nc, in_maps, list(range(8)),
                                          trace=False)
    y = np.stack([res.results[b]["y"] for b in range(B)], axis=0)
    fmem = np.stack([res.results[b]["fmem"] for b in range(B)], axis=0)
    return y.astype(np.float32), fmem.astype(np.float32)
